# revision 29
# baseline (speedup 1.0000x reference)
"""Trainium2 Bass kernel for DPRNN (dropout RNN) — data-parallel over 8 cores.

Model (per batch element b, T=50 steps, I=2, H=20, O=2):
    xp[t] = x[t] @ W_ih.T + b_ih + b_hh
    h[t]  = tanh(xp[t] + h[t-1] @ W_hh.T),  h[-1] = 0
    out[t] = (h[t] * mask[t]) @ W_out.T + b_out

Wire format (minimize bytes at the dispatch boundary — memory regime):
  - x shipped as fp16 (13MB total), weights fp16, output fp16.
  - drop_mask shipped as bit-packed uint8 keep bits (262MB -> 8.2MB);
    expanded on-device with 2 DVE ops per bit position per 5-step block
    (bitwise_and u8->u8, then not_equal u8->f16 strided write).
    The 1/(1-p)=1.25 dropout scale is folded into W_out host-side.

Device strategy (per core, B/8 batch rows):
  - hidden dim on SBUF partitions; G=6 batch groups packed block-diagonally
    (120 of 128 partitions); batch columns split into 3 PSUM-bank chunks
    that form INDEPENDENT recurrence chains (separate h tiles per chunk) so
    the serial t-dependency pipelines across chunks.
  - per timestep+chunk: in-proj matmul + recurrence matmul accumulate in
    PSUM (fp16 operands, f32 accum), ACT tanh(+bias) -> fp16 h chunk, DVE
    mask-mul, out-proj matmul into a PSUM tile at partition offset 32*(t%4);
    per 4 timesteps one DVE copy(+bias, f32->f16) and one tile DMA out.
"""

import numpy as np

B, T, I, H, O = 65536, 50, 2, 20, 2
NCORES = 8
G = 6                      # batch groups packed along partitions
NC = 1368                  # batch columns per group per core (div by 8)
NC8 = NC // 8              # packed mask bytes per group per core
BCORE = G * NC             # 8208 padded batch rows per core
BPAD = NCORES * BCORE      # 65664
PH, PI, PO = G * H, G * I, G * O   # 120, 12, 12
TS = 4                     # timesteps per out-PSUM supergroup
PSTRIDE = 32               # partition offset per timestep within supergroup
PSO_ROWS = TS * PSTRIDE    # 128 (out-proj writes full 32-row stripes)
ODROWS = TS * PO           # 48 dense output rows shipped per supergroup
NGRP = (T + TS - 1) // TS  # 13 output supergroups (12 full + 1 of 2)
TB = 5                     # timesteps per input DMA block
NTB = T // TB              # 10
CHUNKS = [(0, 512), (512, 512), (1024, NC - 1024)]  # psum bank-aligned chunks

_CACHE = {}


def _build_module(repeat=1, mode="full"):
    import concourse.bass as bass
    import concourse.bacc as bacc
    import concourse.tile as tile
    from concourse import mybir

    f16 = mybir.dt.float16
    f32 = mybir.dt.float32
    u8 = mybir.dt.uint8
    TANH = mybir.ActivationFunctionType.Tanh
    COPY = mybir.ActivationFunctionType.Copy
    AND = mybir.AluOpType.bitwise_and
    NEQ = mybir.AluOpType.not_equal

    nc = bacc.Bacc("TRN2", target_bir_lowering=False, debug=False,
                   num_devices=NCORES)

    xT = nc.dram_tensor("xT", [PI, T * NC], f16, kind="ExternalInput")
    maskb = nc.dram_tensor("maskb", [PH, T * NC8], u8,
                           kind="ExternalInput")
    wih = nc.dram_tensor("wih", [PI, PH], f16, kind="ExternalInput")
    whh = nc.dram_tensor("whh", [PH, PH], f16, kind="ExternalInput")
    wout = nc.dram_tensor("wout", [PH, PSTRIDE], f16, kind="ExternalInput")
    bh = nc.dram_tensor("bh", [PH, 1], f32, kind="ExternalInput")
    outd = nc.dram_tensor("outd", [NGRP, ODROWS, NC], f16,
                          kind="ExternalOutput")

    xT_ap, maskb_ap, outd_ap = xT.ap(), maskb.ap(), outd.ap()

    with tile.TileContext(nc) as tc:
        with (
            tc.tile_pool(name="w", bufs=1) as wp,
            tc.tile_pool(name="km", bufs=2) as kp,
            tc.tile_pool(name="h", bufs=4) as hp,
            tc.tile_pool(name="rm", bufs=4) as rp,
            tc.tile_pool(name="osb", bufs=2) as op,
            tc.tile_pool(name="psr", bufs=4, space=bass.MemorySpace.PSUM) as pr,
            tc.tile_pool(name="pso", bufs=1, space=bass.MemorySpace.PSUM) as po,
        ):
            w_ih = wp.tile([PI, PH], f16)
            nc.sync.dma_start(w_ih[:], wih.ap())
            w_hh = wp.tile([PH, PH], f16)
            nc.sync.dma_start(w_hh[:], whh.ap())
            w_out = wp.tile([PH, PSTRIDE], f16)
            nc.sync.dma_start(w_out[:], wout.ap())
            b_h = wp.tile([PH, 1], f32)
            nc.sync.dma_start(b_h[:], bh.ap())
            x_all = wp.tile([PI, T * NC], f16)
            nc.sync.dma_start(x_all[:], xT_ap)
            m_all = wp.tile([PH, T * NC8], u8)
            nc.sync.dma_start(m_all[:], maskb_ap)

            if mode == "dmaonly":
                o_c = wp.tile([ODROWS, NC], f16)
                nc.vector.memset(o_c[:], 0.0)

            for rep in range(repeat):
                h_prev = [None] * len(CHUNKS)
                ps_o = None
                km_b = None
                for t in range(T):
                    grp, t8 = t // TS, t % TS
                    cur_ts = min(TS, T - grp * TS)
                    orows = cur_ts * PSTRIDE
                    q, r = t // TB, t % TB
                    off = r * NC

                    if r == 0 and mode != "dmaonly":
                        km_b = kp.tile([PH, TB * NC], f16, tag="km",
                                       name=f"km_{rep}_{q}")
                        bt = kp.tile([PH, TB * NC8], u8, tag="kmtmp",
                                     name=f"bt_{rep}_{q}")
                        moff = q * TB * NC8
                        for p in range(8):
                            nc.vector.tensor_scalar(
                                bt[:], m_all[:, moff:moff + TB * NC8],
                                1 << p, None, AND)
                            nc.vector.tensor_scalar(
                                km_b[:, p::8], bt[:], 0, None, NEQ)

                    if mode == "dmaonly":
                        if t8 == cur_ts - 1:
                            nc.sync.dma_start(outd_ap[grp, :cur_ts * PO, :],
                                              o_c[:cur_ts * PO, :])
                        continue

                    if t8 == 0:
                        ps_o = [po.tile([orows, 512], f32, tag=f"pso{c}",
                                        name=f"pso{c}_{rep}_{grp}")[:, :n]
                                for c, (s, n) in enumerate(CHUNKS)]

                    for c, (s, n) in enumerate(CHUNKS):
                        ps = pr.tile([PH, 512], f32, tag="psr",
                                     name=f"psr_{rep}_{t}_{c}")[:, :n]
                        nc.tensor.matmul(ps[:], w_ih[:],
                                         x_all[:, t * NC + s: t * NC + s + n],
                                         start=True, stop=(t == 0))
                        if t > 0:
                            nc.tensor.matmul(ps[:], w_hh[:], h_prev[c][:],
                                             start=False, stop=True)
                        h_new = hp.tile([PH, n], f16, tag=f"h{c}",
                                        name=f"h_{rep}_{t}_{c}")
                        nc.scalar.activation(h_new[:], ps[:], TANH,
                                             bias=b_h[:])
                        h_prev[c] = h_new
                        rm = rp.tile([PH, n], f16, tag=f"rm{c}",
                                     name=f"rm_{rep}_{t}_{c}")
                        nc.vector.tensor_mul(rm[:], h_new[:],
                                             km_b[:, off + s: off + s + n])
                        base = t8 * PSTRIDE
                        nc.tensor.matmul(ps_o[c][base:base + PSTRIDE, :],
                                         w_out[:], rm[:],
                                         start=True, stop=True,
                                         tile_position=(0, base))

                    if t8 == cur_ts - 1:
                        o_sb = op.tile([PSO_ROWS, NC], f16, tag="osb",
                                       name=f"osb_{rep}_{grp}")
                        for c, (s, n) in enumerate(CHUNKS):
                            for k in range(cur_ts):
                                dst = o_sb[k * PSTRIDE:k * PSTRIDE + PO,
                                           s:s + n]
                                src = ps_o[c][k * PSTRIDE:k * PSTRIDE + PO, :]
                                if (c + k) % 2 == 0:
                                    nc.scalar.activation(dst, src, COPY)
                                else:
                                    nc.vector.tensor_copy(dst, src)
                        for k in range(cur_ts):
                            nc.sync.dma_start(
                                outd_ap[grp, k * PO:(k + 1) * PO, :],
                                o_sb[k * PSTRIDE:k * PSTRIDE + PO, :])

    nc.compile()
    return nc


def _get_module(repeat=1, mode="full"):
    key = ("nc", repeat, mode)
    if key not in _CACHE:
        _CACHE[key] = _build_module(repeat, mode)
    return _CACHE[key]


def pack_inputs(x, W_ih, W_hh, b_ih, b_hh, W_out, b_out, drop_mask):
    """Host-side shard + layout permute + wire compression."""
    x = np.asarray(x, np.float32)
    drop_mask = np.asarray(drop_mask)
    W_ih = np.asarray(W_ih, np.float32)
    W_hh = np.asarray(W_hh, np.float32)
    W_out = np.asarray(W_out, np.float32)
    b_ih = np.asarray(b_ih, np.float32)
    b_hh = np.asarray(b_hh, np.float32)
    b_out = np.asarray(b_out, np.float32)

    xpad = np.zeros((BPAD, T, I), np.float32)
    xpad[:B] = x
    keep = np.zeros((BPAD, T, H), np.uint8)
    keep[:B] = drop_mask > 0

    # x: [core, G, NC, T, I] -> [core, (G I), (T NC)] fp16 (one flat DMA)
    xr = xpad.reshape(NCORES, G, NC, T, I).transpose(0, 1, 4, 3, 2)
    xT = np.ascontiguousarray(xr).reshape(
        NCORES, PI, T * NC).astype(np.float16)
    # keep bits: [core, (G H), (T NC/8)] packed little-endian along NC
    kr = keep.reshape(NCORES, G, NC, T, H).transpose(0, 3, 1, 4, 2)
    kr = np.ascontiguousarray(kr).reshape(NCORES, T, PH, NC)
    kp = np.packbits(kr, axis=-1, bitorder="little")  # [8, T, PH, NC8]
    maskb = np.ascontiguousarray(kp.transpose(0, 2, 1, 3)).reshape(
        NCORES, PH, T * NC8)

    wih_blk = np.zeros((PI, PH), np.float32)
    whh_blk = np.zeros((PH, PH), np.float32)
    wout_blk = np.zeros((PH, PSTRIDE), np.float32)
    for g in range(G):
        wih_blk[g * I:(g + 1) * I, g * H:(g + 1) * H] = W_ih.T
        whh_blk[g * H:(g + 1) * H, g * H:(g + 1) * H] = W_hh.T
        # dropout inverted scaling 1/(1-0.2) folded into the out projection
        wout_blk[g * H:(g + 1) * H, g * O:(g + 1) * O] = 1.25 * W_out.T
    bh_v = np.tile(b_ih + b_hh, G).reshape(PH, 1).astype(np.float32)

    return [{
        "xT": xT[c].copy(),
        "maskb": maskb[c].copy(),
        "wih": wih_blk.astype(np.float16),
        "whh": whh_blk.astype(np.float16),
        "wout": wout_blk.astype(np.float16),
        "bh": bh_v,
    } for c in range(NCORES)]


def unpack_output(outd_list):
    """outd_list: 8 arrays [NGRP, ODROWS, NC] f16 -> full [B, T, O] f32."""
    o = np.stack([np.asarray(a) for a in outd_list]).astype(np.float32)
    oh = np.empty((NCORES, T, PO, NC), np.float32)
    for t in range(T):
        grp, k = t // TS, t % TS
        oh[:, t] = o[:, grp, k * PO:(k + 1) * PO, :]
    oh = oh.reshape(NCORES, T, G, O, NC).transpose(0, 2, 4, 1, 3)
    return np.ascontiguousarray(oh).reshape(BPAD, T, O)[:B]


def kernel(x, W_ih, W_hh, b_ih, b_hh, W_out, b_out, drop_mask):
    from concourse import bass_utils
    nc = _get_module()
    in_maps = pack_inputs(x, W_ih, W_hh, b_ih, b_hh, W_out, b_out, drop_mask)
    res = bass_utils.run_bass_kernel_spmd(nc, in_maps,
                                          core_ids=list(range(NCORES)))
    out = unpack_output([r["outd"] for r in res.results])
    # b_out is folded in on the host (the device ships biasless fp16 sums)
    out += np.asarray(b_out, np.float32)
    return out


# revision 30
# speedup vs baseline: 1.1894x; 1.1894x over previous
"""Trainium2 Bass kernel for DPRNN (dropout RNN) — data-parallel over 8 cores.

Model (per batch element b, T=50 steps, I=2, H=20, O=2):
    xp[t] = x[t] @ W_ih.T + b_ih + b_hh
    h[t]  = tanh(xp[t] + h[t-1] @ W_hh.T),  h[-1] = 0
    out[t] = (h[t] * mask[t]) @ W_out.T + b_out

Wire format (minimize bytes at the dispatch boundary — memory regime):
  - x shipped as fp16 (13MB total), weights fp16, output fp16.
  - drop_mask shipped as bit-packed uint8 keep bits (262MB -> 8.2MB);
    expanded on-device with 2 DVE ops per bit position per 5-step block
    (bitwise_and u8->u8, then not_equal u8->f16 strided write).
    The 1/(1-p)=1.25 dropout scale is folded into W_out host-side.

Device strategy (per core, B/8 batch rows):
  - hidden dim on SBUF partitions; G=6 batch groups packed block-diagonally
    (120 of 128 partitions); batch columns split into 3 PSUM-bank chunks
    that form INDEPENDENT recurrence chains (separate h tiles per chunk) so
    the serial t-dependency pipelines across chunks.
  - per timestep+chunk: in-proj matmul + recurrence matmul accumulate in
    PSUM (fp16 operands, f32 accum), ACT tanh(+bias) -> fp16 h chunk, DVE
    mask-mul, out-proj matmul into a PSUM tile at partition offset 32*(t%4);
    per 4 timesteps one DVE copy(+bias, f32->f16) and one tile DMA out.
"""

import numpy as np

B, T, I, H, O = 65536, 50, 2, 20, 2
NCORES = 8
G = 6                      # batch groups packed along partitions
NC = 1368                  # batch columns per group per core (div by 8)
NC8 = NC // 8              # packed mask bytes per group per core
BCORE = G * NC             # 8208 padded batch rows per core
BPAD = NCORES * BCORE      # 65664
PH, PI, PO = G * H, G * I, G * O   # 120, 12, 12
TS = 4                     # timesteps per out-PSUM supergroup
PSTRIDE = 32               # partition offset per timestep within supergroup
PSO_ROWS = TS * PSTRIDE    # 128 (out-proj writes full 32-row stripes)
ODROWS = TS * PO           # 48 dense output rows shipped per supergroup
NGRP = (T + TS - 1) // TS  # 13 output supergroups (12 full + 1 of 2)
TB = 5                     # timesteps per input DMA block
NTB = T // TB              # 10
CHUNKS = [(0, 512), (512, 512), (1024, NC - 1024)]  # psum bank-aligned chunks

_CACHE = {}


def _build_module(repeat=1, mode="full"):
    import concourse.bass as bass
    import concourse.bacc as bacc
    import concourse.tile as tile
    from concourse import mybir

    f16 = mybir.dt.float16
    f32 = mybir.dt.float32
    u8 = mybir.dt.uint8
    TANH = mybir.ActivationFunctionType.Tanh
    COPY = mybir.ActivationFunctionType.Copy
    AND = mybir.AluOpType.bitwise_and
    NEQ = mybir.AluOpType.not_equal

    nc = bacc.Bacc("TRN2", target_bir_lowering=False, debug=False,
                   num_devices=NCORES)

    xT = nc.dram_tensor("xT", [PI, T * NC], f16, kind="ExternalInput")
    maskb = nc.dram_tensor("maskb", [PH, T * NC8], u8,
                           kind="ExternalInput")
    wih = nc.dram_tensor("wih", [PI, PH], f16, kind="ExternalInput")
    whh = nc.dram_tensor("whh", [PH, PH], f16, kind="ExternalInput")
    wout = nc.dram_tensor("wout", [PH, PSTRIDE], f16, kind="ExternalInput")
    bh = nc.dram_tensor("bh", [PH, 1], f32, kind="ExternalInput")
    outd = nc.dram_tensor("outd", [NGRP, ODROWS, NC], f16,
                          kind="ExternalOutput")

    xT_ap, maskb_ap, outd_ap = xT.ap(), maskb.ap(), outd.ap()

    with tile.TileContext(nc) as tc:
        with (
            tc.tile_pool(name="w", bufs=1) as wp,
            tc.tile_pool(name="km", bufs=2) as kp,
            tc.tile_pool(name="h", bufs=4) as hp,
            tc.tile_pool(name="rm", bufs=4) as rp,
            tc.tile_pool(name="osb", bufs=2) as op,
            tc.tile_pool(name="psr", bufs=4, space=bass.MemorySpace.PSUM) as pr,
            tc.tile_pool(name="pso", bufs=1, space=bass.MemorySpace.PSUM) as po,
        ):
            w_ih = wp.tile([PI, PH], f16)
            nc.sync.dma_start(w_ih[:], wih.ap())
            w_hh = wp.tile([PH, PH], f16)
            nc.sync.dma_start(w_hh[:], whh.ap())
            w_out = wp.tile([PH, PSTRIDE], f16)
            nc.sync.dma_start(w_out[:], wout.ap())
            b_h = wp.tile([PH, 1], f32)
            nc.sync.dma_start(b_h[:], bh.ap())
            # split head DMA so block-0 compute starts before the tail lands
            x_all = wp.tile([PI, T * NC], f16)
            nc.sync.dma_start(x_all[:, :TB * NC], xT_ap[:, :TB * NC])
            nc.sync.dma_start(x_all[:, TB * NC:], xT_ap[:, TB * NC:])
            m_all = wp.tile([PH, T * NC8], u8)
            nc.sync.dma_start(m_all[:, :TB * NC8], maskb_ap[:, :TB * NC8])
            nc.sync.dma_start(m_all[:, TB * NC8:], maskb_ap[:, TB * NC8:])

            if mode == "dmaonly":
                o_c = wp.tile([ODROWS, NC], f16)
                nc.vector.memset(o_c[:], 0.0)

            for rep in range(repeat):
                h_prev = [None] * len(CHUNKS)
                ps_o = None
                km_b = None
                for t in range(T):
                    grp, t8 = t // TS, t % TS
                    cur_ts = min(TS, T - grp * TS)
                    orows = cur_ts * PSTRIDE
                    q, r = t // TB, t % TB
                    off = r * NC

                    if r == 0 and mode != "dmaonly":
                        km_b = kp.tile([PH, TB * NC], f16, tag="km",
                                       name=f"km_{rep}_{q}")
                        bt = kp.tile([PH, TB * NC8], u8, tag="kmtmp",
                                     name=f"bt_{rep}_{q}")
                        moff = q * TB * NC8
                        for p in range(8):
                            nc.vector.tensor_scalar(
                                bt[:], m_all[:, moff:moff + TB * NC8],
                                1 << p, None, AND)
                            nc.vector.tensor_scalar(
                                km_b[:, p::8], bt[:], 0, None, NEQ)

                    if mode == "dmaonly":
                        if t8 == cur_ts - 1:
                            nc.sync.dma_start(outd_ap[grp, :cur_ts * PO, :],
                                              o_c[:cur_ts * PO, :])
                        continue

                    if t8 == 0:
                        ps_o = [po.tile([orows, 512], f32, tag=f"pso{c}",
                                        name=f"pso{c}_{rep}_{grp}")[:, :n]
                                for c, (s, n) in enumerate(CHUNKS)]

                    for c, (s, n) in enumerate(CHUNKS):
                        ps = pr.tile([PH, 512], f32, tag="psr",
                                     name=f"psr_{rep}_{t}_{c}")[:, :n]
                        nc.tensor.matmul(ps[:], w_ih[:],
                                         x_all[:, t * NC + s: t * NC + s + n],
                                         start=True, stop=(t == 0))
                        if t > 0:
                            nc.tensor.matmul(ps[:], w_hh[:], h_prev[c][:],
                                             start=False, stop=True)
                        h_new = hp.tile([PH, n], f16, tag=f"h{c}",
                                        name=f"h_{rep}_{t}_{c}")
                        nc.scalar.activation(h_new[:], ps[:], TANH,
                                             bias=b_h[:])
                        h_prev[c] = h_new
                        rm = rp.tile([PH, n], f16, tag=f"rm{c}",
                                     name=f"rm_{rep}_{t}_{c}")
                        nc.vector.tensor_mul(rm[:], h_new[:],
                                             km_b[:, off + s: off + s + n])
                        base = t8 * PSTRIDE
                        nc.tensor.matmul(ps_o[c][base:base + PSTRIDE, :],
                                         w_out[:], rm[:],
                                         start=True, stop=True,
                                         tile_position=(0, base))

                    if t8 == cur_ts - 1:
                        o_sb = op.tile([PSO_ROWS, NC], f16, tag="osb",
                                       name=f"osb_{rep}_{grp}")
                        for c, (s, n) in enumerate(CHUNKS):
                            for k in range(cur_ts):
                                dst = o_sb[k * PSTRIDE:k * PSTRIDE + PO,
                                           s:s + n]
                                src = ps_o[c][k * PSTRIDE:k * PSTRIDE + PO, :]
                                if (c + k) % 2 == 0:
                                    nc.scalar.activation(dst, src, COPY)
                                else:
                                    nc.vector.tensor_copy(dst, src)
                        for k in range(cur_ts):
                            nc.sync.dma_start(
                                outd_ap[grp, k * PO:(k + 1) * PO, :],
                                o_sb[k * PSTRIDE:k * PSTRIDE + PO, :])

    nc.compile()
    return nc


def _get_module(repeat=1, mode="full"):
    key = ("nc", repeat, mode)
    if key not in _CACHE:
        _CACHE[key] = _build_module(repeat, mode)
    return _CACHE[key]


def pack_inputs(x, W_ih, W_hh, b_ih, b_hh, W_out, b_out, drop_mask):
    """Host-side shard + layout permute + wire compression."""
    x = np.asarray(x, np.float32)
    drop_mask = np.asarray(drop_mask)
    W_ih = np.asarray(W_ih, np.float32)
    W_hh = np.asarray(W_hh, np.float32)
    W_out = np.asarray(W_out, np.float32)
    b_ih = np.asarray(b_ih, np.float32)
    b_hh = np.asarray(b_hh, np.float32)
    b_out = np.asarray(b_out, np.float32)

    xpad = np.zeros((BPAD, T, I), np.float32)
    xpad[:B] = x
    keep = np.zeros((BPAD, T, H), np.uint8)
    keep[:B] = drop_mask > 0

    # x: [core, G, NC, T, I] -> [core, (G I), (T NC)] fp16 (one flat DMA)
    xr = xpad.reshape(NCORES, G, NC, T, I).transpose(0, 1, 4, 3, 2)
    xT = np.ascontiguousarray(xr).reshape(
        NCORES, PI, T * NC).astype(np.float16)
    # keep bits: [core, (G H), (T NC/8)] packed little-endian along NC
    kr = keep.reshape(NCORES, G, NC, T, H).transpose(0, 3, 1, 4, 2)
    kr = np.ascontiguousarray(kr).reshape(NCORES, T, PH, NC)
    kp = np.packbits(kr, axis=-1, bitorder="little")  # [8, T, PH, NC8]
    maskb = np.ascontiguousarray(kp.transpose(0, 2, 1, 3)).reshape(
        NCORES, PH, T * NC8)

    wih_blk = np.zeros((PI, PH), np.float32)
    whh_blk = np.zeros((PH, PH), np.float32)
    wout_blk = np.zeros((PH, PSTRIDE), np.float32)
    for g in range(G):
        wih_blk[g * I:(g + 1) * I, g * H:(g + 1) * H] = W_ih.T
        whh_blk[g * H:(g + 1) * H, g * H:(g + 1) * H] = W_hh.T
        # dropout inverted scaling 1/(1-0.2) folded into the out projection
        wout_blk[g * H:(g + 1) * H, g * O:(g + 1) * O] = 1.25 * W_out.T
    bh_v = np.tile(b_ih + b_hh, G).reshape(PH, 1).astype(np.float32)

    return [{
        "xT": xT[c].copy(),
        "maskb": maskb[c].copy(),
        "wih": wih_blk.astype(np.float16),
        "whh": whh_blk.astype(np.float16),
        "wout": wout_blk.astype(np.float16),
        "bh": bh_v,
    } for c in range(NCORES)]


def unpack_output(outd_list):
    """outd_list: 8 arrays [NGRP, ODROWS, NC] f16 -> full [B, T, O] f32."""
    o = np.stack([np.asarray(a) for a in outd_list]).astype(np.float32)
    oh = np.empty((NCORES, T, PO, NC), np.float32)
    for t in range(T):
        grp, k = t // TS, t % TS
        oh[:, t] = o[:, grp, k * PO:(k + 1) * PO, :]
    oh = oh.reshape(NCORES, T, G, O, NC).transpose(0, 2, 4, 1, 3)
    return np.ascontiguousarray(oh).reshape(BPAD, T, O)[:B]


def kernel(x, W_ih, W_hh, b_ih, b_hh, W_out, b_out, drop_mask):
    from concourse import bass_utils
    nc = _get_module()
    in_maps = pack_inputs(x, W_ih, W_hh, b_ih, b_hh, W_out, b_out, drop_mask)
    res = bass_utils.run_bass_kernel_spmd(nc, in_maps,
                                          core_ids=list(range(NCORES)))
    out = unpack_output([r["outd"] for r in res.results])
    # b_out is folded in on the host (the device ships biasless fp16 sums)
    out += np.asarray(b_out, np.float32)
    return out


# revision 31
# speedup vs baseline: 1.1957x; 1.0053x over previous
"""Trainium2 Bass kernel for DPRNN (dropout RNN) — data-parallel over 8 cores.

Model (per batch element b, T=50 steps, I=2, H=20, O=2):
    xp[t] = x[t] @ W_ih.T + b_ih + b_hh
    h[t]  = tanh(xp[t] + h[t-1] @ W_hh.T),  h[-1] = 0
    out[t] = (h[t] * mask[t]) @ W_out.T + b_out

Wire format (minimize bytes at the dispatch boundary — memory regime):
  - x shipped as fp16 (13MB total), weights fp16, output fp16.
  - drop_mask shipped as bit-packed uint8 keep bits (262MB -> 8.2MB);
    expanded on-device with 2 DVE ops per bit position per 5-step block
    (bitwise_and u8->u8, then not_equal u8->f16 strided write).
    The 1/(1-p)=1.25 dropout scale is folded into W_out host-side.

Device strategy (per core, B/8 batch rows):
  - hidden dim on SBUF partitions; G=6 batch groups packed block-diagonally
    (120 of 128 partitions); batch columns split into 3 PSUM-bank chunks
    that form INDEPENDENT recurrence chains (separate h tiles per chunk) so
    the serial t-dependency pipelines across chunks.
  - per timestep+chunk: in-proj matmul + recurrence matmul accumulate in
    PSUM (fp16 operands, f32 accum), ACT tanh(+bias) -> fp16 h chunk, DVE
    mask-mul, out-proj matmul into a PSUM tile at partition offset 32*(t%4);
    per 4 timesteps: cast copies (f32->f16, alternating ACT/DVE) densify
    the 12 valid rows of each 32-stripe, then 4 small DMAs ship them
    (engine APs need 32-aligned partition bases; DMA does not).
    b_out is added on the host after unpack; x/mask load as single flat
    DMAs split head+tail so block-0 compute overlaps the tail transfer.
"""

import numpy as np

B, T, I, H, O = 65536, 50, 2, 20, 2
NCORES = 8
G = 6                      # batch groups packed along partitions
NC = 1368                  # batch columns per group per core (div by 8)
NC8 = NC // 8              # packed mask bytes per group per core
BCORE = G * NC             # 8208 padded batch rows per core
BPAD = NCORES * BCORE      # 65664
PH, PI, PO = G * H, G * I, G * O   # 120, 12, 12
TS = 4                     # timesteps per out-PSUM supergroup
PSTRIDE = 32               # partition offset per timestep within supergroup
PSO_ROWS = TS * PSTRIDE    # 128 (out-proj writes full 32-row stripes)
ODROWS = TS * PO           # 48 dense output rows shipped per supergroup
NGRP = (T + TS - 1) // TS  # 13 output supergroups (12 full + 1 of 2)
TB = 5                     # timesteps per input DMA block
NTB = T // TB              # 10
CHUNKS = [(0, 512), (512, 512), (1024, NC - 1024)]  # psum bank-aligned chunks

_CACHE = {}


def _build_module(repeat=1, mode="full"):
    import concourse.bass as bass
    import concourse.bacc as bacc
    import concourse.tile as tile
    from concourse import mybir

    f16 = mybir.dt.float16
    f32 = mybir.dt.float32
    u8 = mybir.dt.uint8
    TANH = mybir.ActivationFunctionType.Tanh
    COPY = mybir.ActivationFunctionType.Copy
    AND = mybir.AluOpType.bitwise_and
    NEQ = mybir.AluOpType.not_equal

    nc = bacc.Bacc("TRN2", target_bir_lowering=False, debug=False,
                   num_devices=NCORES)

    xT = nc.dram_tensor("xT", [PI, T * NC], f16, kind="ExternalInput")
    maskb = nc.dram_tensor("maskb", [PH, T * NC8], u8,
                           kind="ExternalInput")
    wih = nc.dram_tensor("wih", [PI, PH], f16, kind="ExternalInput")
    whh = nc.dram_tensor("whh", [PH, PH], f16, kind="ExternalInput")
    wout = nc.dram_tensor("wout", [PH, PSTRIDE], f16, kind="ExternalInput")
    bh = nc.dram_tensor("bh", [PH, 1], f32, kind="ExternalInput")
    outd = nc.dram_tensor("outd", [NGRP, ODROWS, NC], f16,
                          kind="ExternalOutput")

    xT_ap, maskb_ap, outd_ap = xT.ap(), maskb.ap(), outd.ap()

    with tile.TileContext(nc) as tc:
        with (
            tc.tile_pool(name="w", bufs=1) as wp,
            tc.tile_pool(name="km", bufs=2) as kp,
            tc.tile_pool(name="h", bufs=4) as hp,
            tc.tile_pool(name="rm", bufs=4) as rp,
            tc.tile_pool(name="osb", bufs=2) as op,
            tc.tile_pool(name="psr", bufs=4, space=bass.MemorySpace.PSUM) as pr,
            tc.tile_pool(name="pso", bufs=1, space=bass.MemorySpace.PSUM) as po,
        ):
            w_ih = wp.tile([PI, PH], f16)
            nc.sync.dma_start(w_ih[:], wih.ap())
            w_hh = wp.tile([PH, PH], f16)
            nc.sync.dma_start(w_hh[:], whh.ap())
            w_out = wp.tile([PH, PSTRIDE], f16)
            nc.sync.dma_start(w_out[:], wout.ap())
            b_h = wp.tile([PH, 1], f32)
            nc.sync.dma_start(b_h[:], bh.ap())
            # split head DMA so block-0 compute starts before the tail lands
            x_all = wp.tile([PI, T * NC], f16)
            nc.sync.dma_start(x_all[:, :TB * NC], xT_ap[:, :TB * NC])
            nc.sync.dma_start(x_all[:, TB * NC:], xT_ap[:, TB * NC:])
            m_all = wp.tile([PH, T * NC8], u8)
            nc.sync.dma_start(m_all[:, :TB * NC8], maskb_ap[:, :TB * NC8])
            nc.sync.dma_start(m_all[:, TB * NC8:], maskb_ap[:, TB * NC8:])

            if mode == "dmaonly":
                o_c = wp.tile([ODROWS, NC], f16)
                nc.vector.memset(o_c[:], 0.0)

            for rep in range(repeat):
                h_prev = [None] * len(CHUNKS)
                ps_o = None
                km_b = None
                for t in range(T):
                    grp, t8 = t // TS, t % TS
                    cur_ts = min(TS, T - grp * TS)
                    orows = cur_ts * PSTRIDE
                    q, r = t // TB, t % TB
                    off = r * NC

                    if r == 0 and mode != "dmaonly":
                        km_b = kp.tile([PH, TB * NC], f16, tag="km",
                                       name=f"km_{rep}_{q}")
                        bt = kp.tile([PH, TB * NC8], u8, tag="kmtmp",
                                     name=f"bt_{rep}_{q}")
                        moff = q * TB * NC8
                        for p in range(8):
                            nc.vector.tensor_scalar(
                                bt[:], m_all[:, moff:moff + TB * NC8],
                                1 << p, None, AND)
                            nc.vector.tensor_scalar(
                                km_b[:, p::8], bt[:], 0, None, NEQ)

                    if mode == "dmaonly":
                        if t8 == cur_ts - 1:
                            nc.sync.dma_start(outd_ap[grp, :cur_ts * PO, :],
                                              o_c[:cur_ts * PO, :])
                        continue

                    if t8 == 0:
                        ps_o = [po.tile([orows, 512], f32, tag=f"pso{c}",
                                        name=f"pso{c}_{rep}_{grp}")[:, :n]
                                for c, (s, n) in enumerate(CHUNKS)]

                    for c, (s, n) in enumerate(CHUNKS):
                        ps = pr.tile([PH, 512], f32, tag="psr",
                                     name=f"psr_{rep}_{t}_{c}")[:, :n]
                        nc.tensor.matmul(ps[:], w_ih[:],
                                         x_all[:, t * NC + s: t * NC + s + n],
                                         start=True, stop=(t == 0))
                        if t > 0:
                            nc.tensor.matmul(ps[:], w_hh[:], h_prev[c][:],
                                             start=False, stop=True)
                        h_new = hp.tile([PH, n], f16, tag=f"h{c}",
                                        name=f"h_{rep}_{t}_{c}")
                        nc.scalar.activation(h_new[:], ps[:], TANH,
                                             bias=b_h[:])
                        h_prev[c] = h_new
                        rm = rp.tile([PH, n], f16, tag=f"rm{c}",
                                     name=f"rm_{rep}_{t}_{c}")
                        nc.vector.tensor_mul(rm[:], h_new[:],
                                             km_b[:, off + s: off + s + n])
                        base = t8 * PSTRIDE
                        nc.tensor.matmul(ps_o[c][base:base + PSTRIDE, :],
                                         w_out[:], rm[:],
                                         start=True, stop=True,
                                         tile_position=(0, base))

                    if t8 == cur_ts - 1:
                        o_sb = op.tile([PSO_ROWS, NC], f16, tag="osb",
                                       name=f"osb_{rep}_{grp}")
                        for c, (s, n) in enumerate(CHUNKS):
                            for k in range(cur_ts):
                                dst = o_sb[k * PSTRIDE:k * PSTRIDE + PO,
                                           s:s + n]
                                src = ps_o[c][k * PSTRIDE:k * PSTRIDE + PO, :]
                                if (c + k) % 2 == 0:
                                    nc.scalar.activation(dst, src, COPY)
                                else:
                                    nc.vector.tensor_copy(dst, src)
                        for k in range(cur_ts):
                            nc.sync.dma_start(
                                outd_ap[grp, k * PO:(k + 1) * PO, :],
                                o_sb[k * PSTRIDE:k * PSTRIDE + PO, :])

    nc.compile()
    return nc


def _get_module(repeat=1, mode="full"):
    key = ("nc", repeat, mode)
    if key not in _CACHE:
        _CACHE[key] = _build_module(repeat, mode)
    return _CACHE[key]


def pack_inputs(x, W_ih, W_hh, b_ih, b_hh, W_out, b_out, drop_mask):
    """Host-side shard + layout permute + wire compression."""
    x = np.asarray(x, np.float32)
    drop_mask = np.asarray(drop_mask)
    W_ih = np.asarray(W_ih, np.float32)
    W_hh = np.asarray(W_hh, np.float32)
    W_out = np.asarray(W_out, np.float32)
    b_ih = np.asarray(b_ih, np.float32)
    b_hh = np.asarray(b_hh, np.float32)
    b_out = np.asarray(b_out, np.float32)

    xpad = np.zeros((BPAD, T, I), np.float32)
    xpad[:B] = x
    keep = np.zeros((BPAD, T, H), np.uint8)
    keep[:B] = drop_mask > 0

    # x: [core, G, NC, T, I] -> [core, (G I), (T NC)] fp16 (one flat DMA)
    xr = xpad.reshape(NCORES, G, NC, T, I).transpose(0, 1, 4, 3, 2)
    xT = np.ascontiguousarray(xr).reshape(
        NCORES, PI, T * NC).astype(np.float16)
    # keep bits: [core, (G H), (T NC/8)] packed little-endian along NC
    kr = keep.reshape(NCORES, G, NC, T, H).transpose(0, 3, 1, 4, 2)
    kr = np.ascontiguousarray(kr).reshape(NCORES, T, PH, NC)
    kp = np.packbits(kr, axis=-1, bitorder="little")  # [8, T, PH, NC8]
    maskb = np.ascontiguousarray(kp.transpose(0, 2, 1, 3)).reshape(
        NCORES, PH, T * NC8)

    wih_blk = np.zeros((PI, PH), np.float32)
    whh_blk = np.zeros((PH, PH), np.float32)
    wout_blk = np.zeros((PH, PSTRIDE), np.float32)
    for g in range(G):
        wih_blk[g * I:(g + 1) * I, g * H:(g + 1) * H] = W_ih.T
        whh_blk[g * H:(g + 1) * H, g * H:(g + 1) * H] = W_hh.T
        # dropout inverted scaling 1/(1-0.2) folded into the out projection
        wout_blk[g * H:(g + 1) * H, g * O:(g + 1) * O] = 1.25 * W_out.T
    bh_v = np.tile(b_ih + b_hh, G).reshape(PH, 1).astype(np.float32)

    return [{
        "xT": xT[c].copy(),
        "maskb": maskb[c].copy(),
        "wih": wih_blk.astype(np.float16),
        "whh": whh_blk.astype(np.float16),
        "wout": wout_blk.astype(np.float16),
        "bh": bh_v,
    } for c in range(NCORES)]


def unpack_output(outd_list):
    """outd_list: 8 arrays [NGRP, ODROWS, NC] f16 -> full [B, T, O] f32."""
    o = np.stack([np.asarray(a) for a in outd_list]).astype(np.float32)
    oh = np.empty((NCORES, T, PO, NC), np.float32)
    for t in range(T):
        grp, k = t // TS, t % TS
        oh[:, t] = o[:, grp, k * PO:(k + 1) * PO, :]
    oh = oh.reshape(NCORES, T, G, O, NC).transpose(0, 2, 4, 1, 3)
    return np.ascontiguousarray(oh).reshape(BPAD, T, O)[:B]


def kernel(x, W_ih, W_hh, b_ih, b_hh, W_out, b_out, drop_mask):
    from concourse import bass_utils
    nc = _get_module()
    in_maps = pack_inputs(x, W_ih, W_hh, b_ih, b_hh, W_out, b_out, drop_mask)
    res = bass_utils.run_bass_kernel_spmd(nc, in_maps,
                                          core_ids=list(range(NCORES)))
    out = unpack_output([r["outd"] for r in res.results])
    # b_out is folded in on the host (the device ships biasless fp16 sums)
    out += np.asarray(b_out, np.float32)
    return out
